# revision 20
# baseline (speedup 1.0000x reference)
"""DGMNet forward pass on 8 Trainium2 NeuronCores.

Data-parallel: the 131072-point batch is split into 8 shards of 16384; all
weights are replicated and resident in SBUF for the whole kernel.

Per-core layout is feature-major ("transposed"): every activation tensor is
kept as [128 nodes (partitions) x free] tiles, so the matmuls chain without
any on-chip transposes:

    H^T = W^T @ X^T   -> out = lhsT.T @ rhs with lhsT = W[k, m], rhs = X^T

Matmuls run in bf16 (measured ~19% faster than fp32r on HW at rel-err
5e-3, well inside the 2e-2 gate). Weights and X are SHIPPED as bf16,
which (a) halves host->device upload and (b) lets weight DMA land directly
in its SBUF tile: no staging pool, no rounding copies, so PE starts as
soon as W_in + the first X tile land instead of draining a 13 MB preload.

Biases ride the matmuls: X ships with a constant ones row appended
([5, B]) and each input-side weight ships with its bias row appended
([5, n]), so the K=5 X-projection matmul deposits W^T x + b into PSUM for
free. With no per-chunk ACT bias left, each gate's 4 PSUM banks are
written by one [128, 2048] accumulation tile and drained by a SINGLE wide
ScalarE activation — 4x fewer ACT dispatches and semaphore hops. PSUM is
organized as one tag of 2 rotating 4-bank tiles (the full 8 banks); the
two interleaved batch-tiles alternate buffers, so the PE FIFO never waits
more than one ACT drain behind.

silu(x) is computed as x*sigmoid(x) (ACT sigmoid + one fused VectorE op)
so ACT only ever evaluates Sigmoid/Tanh — both live in one activation
table, avoiding the ~1.3us table reload a Silu would force per switch.
o2 = silu(g @ Wo2 + bo2) keeps its bias in the ACT/fused ops (per-chunk,
since a bias row cannot ride g), so it drains per-bank as before.

All elementwise ops run on VectorE (2-byte dtypes at 2x throughput);
offloading them to GpSimd measured ~5% slower end-to-end on HW (software
Q7 implementation), so ew="dve" is the default with "pool" as a knob.

Two batch-tiles are processed interleaved phase-by-phase so one tile's
ACT/DVE tail at each layer boundary hides behind the other tile's matmuls.

kernel() holds one jitted shard_map executable per build and reuses it
across calls (the NEFF stays loaded), so a warm call is upload + execute
instead of a fresh trace/lower/compile each time.
"""

import numpy as np

import concourse.bass as bass
import concourse.mybir as mybir
import concourse.tile as tile
from concourse import bacc
from concourse.bass import ds, ts
from concourse.bass_utils import run_bass_kernel_spmd

N_CORES = 8
BATCH = 131072
B_CORE = BATCH // N_CORES  # 16384
B_TILE = 512
WB = 4 * B_TILE  # wide tile: 4 PSUM banks / 4 node-chunks
N_TILES = B_CORE // B_TILE  # 32
L = 3  # layers
NN = 512  # nodes
D = 4  # input dim
DA = D + 1  # augmented with the ones/bias row
NCH = NN // 128  # node chunks of 128

F32 = mybir.dt.float32
BF16 = mybir.dt.bfloat16
AF = mybir.ActivationFunctionType
ALU = mybir.AluOpType


def build_kernel(n_passes: int = 1, mm_dtype: str = "bf16",
                 no_mm: bool = False, n_tiles: int = N_TILES,
                 loop_mode: str = "static", staggered: bool = False,
                 ew: str = "dve", agran: int = 2, nway: int = 2):
    """Build the per-core Bass program. n_passes > 1 repeats the whole
    compute (for timing); output is identical. no_mm is a timing-ablation
    knob that produces WRONG output.
    X is expected HOST-TRANSPOSED and ones-augmented as [DA, B_CORE] bf16;
    input-side weights ship bias-augmented as [DA, n] bf16."""
    assert mm_dtype == "bf16", mm_dtype
    MMDT = BF16
    nc = bacc.Bacc("TRN2", target_bir_lowering=False, debug=False, num_devices=N_CORES)

    X = nc.dram_tensor("X", [DA, B_CORE], MMDT, kind="ExternalInput")
    W_in = nc.dram_tensor("W_in", [DA, NN], MMDT, kind="ExternalInput")
    Wf = nc.dram_tensor("Wf", [L, DA, NN], MMDT, kind="ExternalInput")
    Uf = nc.dram_tensor("Uf", [L, NN, NN], MMDT, kind="ExternalInput")
    Wu = nc.dram_tensor("Wu", [L, DA, NN], MMDT, kind="ExternalInput")
    Uu = nc.dram_tensor("Uu", [L, NN, NN], MMDT, kind="ExternalInput")
    Wo1 = nc.dram_tensor("Wo1", [L, DA, NN], MMDT, kind="ExternalInput")
    Uo1 = nc.dram_tensor("Uo1", [L, NN, NN], MMDT, kind="ExternalInput")
    Wo2 = nc.dram_tensor("Wo2", [L, NN, NN], MMDT, kind="ExternalInput")
    bo2 = nc.dram_tensor("bo2", [L, 1, NN], MMDT, kind="ExternalInput")
    W_out = nc.dram_tensor("W_out", [NN, 1], MMDT, kind="ExternalInput")
    b_out = nc.dram_tensor("b_out", [1, 1], F32, kind="ExternalInput")
    out = nc.dram_tensor("out", [1, B_CORE], F32, kind="ExternalOutput")

    from contextlib import ExitStack
    with tile.TileContext(nc) as tc, ExitStack() as stack:
        wpool = stack.enter_context(tc.tile_pool(name="weights", bufs=1))
        xt_pool = stack.enter_context(tc.tile_pool(name="xt", bufs=2))
        h_pool = stack.enter_context(tc.tile_pool(name="h", bufs=1))
        g_pool = stack.enter_context(tc.tile_pool(name="gates", bufs=1))
        ps_pool = stack.enter_context(tc.tile_pool(name="psum", bufs=2, space="PSUM"))
        o_pool = stack.enter_context(tc.tile_pool(name="out", bufs=2))

        ew_eng = nc.gpsimd if ew == "pool" else nc.vector

        # Weights DMA straight into their resident bf16 tiles, emitted in
        # consumption order (win, then per-layer u, o1, f, o2) on the SP
        # HWDGE queue; X/out traffic uses the Activation HWDGE queue, so
        # the first tiles' X never queues behind the weight preload and PE
        # starts as soon as W_in lands.
        def load_w(src, tag):
            """[DA, NN] input-side weight with bias row."""
            t = wpool.tile([DA, NN], MMDT, tag=tag, name=tag)
            nc.sync.dma_start(t[:], src)
            return t

        def load_u(src, tag):
            """U-type [512, 512] -> [128, (ko n)]: lhsT chunk (ko, m) is
            [:, ko*NN + m*128 ...]."""
            t = wpool.tile([128, NCH * NN], MMDT, tag=tag, name=tag)
            nc.sync.dma_start(t[:].rearrange("p (ko n) -> p ko n", ko=NCH),
                              src.rearrange("(ko p) n -> p ko n", p=128))
            return t

        def load_b(src, tag):
            """bias [1, NN] kept row-major: [1, m*128+p] slices feed the
            K=1 bias matmul (lhsT [1, 128] per chunk, rhs = ones row)."""
            t = wpool.tile([1, NN], MMDT, tag=tag, name=tag)
            nc.sync.dma_start(t[:], src)
            return t

        win_sb = load_w(W_in[:, :], "win")
        wf_sb, wu_sb, wo1_sb = [None] * L, [None] * L, [None] * L
        uf_sb, uu_sb, uo1_sb, wo2_sb = [None] * L, [None] * L, [None] * L, [None] * L
        bo2_sb = [None] * L

        def load_layer_w(i):
            wu_sb[i] = load_w(Wu[i], f"wu{i}")
            uu_sb[i] = load_u(Uu[i], f"uu{i}")
            wo1_sb[i] = load_w(Wo1[i], f"wo1{i}")
            uo1_sb[i] = load_u(Uo1[i], f"uo1{i}")
            wf_sb[i] = load_w(Wf[i], f"wf{i}")
            uf_sb[i] = load_u(Uf[i], f"uf{i}")
            wo2_sb[i] = load_u(Wo2[i], f"wo2{i}")
            bo2_sb[i] = load_b(bo2[i], f"bo2{i}")

        for i in range(L):
            load_layer_w(i)
        wout_sb = wpool.tile([128, NCH], MMDT, tag="wout", name="wout")
        nc.sync.dma_start(wout_sb[:].rearrange("p (k o) -> p k o", o=1),
                          W_out.rearrange("(ko p) one -> p ko one", p=128))
        bout_sb = wpool.tile([1, 1], F32, tag="bout", name="bout")
        nc.sync.dma_start(bout_sb[:], b_out[:, :])

        gates = (
            (wu_sb, uu_sb, AF.Sigmoid, "u"),
            (wo1_sb, uo1_sb, AF.Tanh, "o1"),
            (wf_sb, uf_sb, AF.Sigmoid, "f"),
        )

        def mm(psum, lhsT, rhs, start, stop):
            if no_mm:
                if start:
                    nc.vector.memset(psum, 0.0)
                return
            nc.tensor.matmul(psum, lhsT, rhs, start=start, stop=stop)

        def bps_tile(name):
            """One 4-bank PSUM accumulation tile; the single tag rotates
            through 2 buffers = all 8 banks."""
            return ps_pool.tile([128, WB], F32, tag="bps", name=name)

        def gate_mms(ps, w_sb, u_sb, xt, h):
            """All 20 matmuls of one gate: per node-chunk m, the K=5
            X+bias projection then 4 K=128 H-chunk accumulations, into
            PSUM bank m of the wide tile."""
            for m in range(NCH):
                sub = ps[:, ds(m * B_TILE, B_TILE)]
                mm(sub, w_sb[:, ts(m, 128)], xt[:], start=True, stop=False)
                for k in range(NCH):
                    mm(sub, u_sb[:, k * NN + m * 128: k * NN + (m + 1) * 128],
                       h[:, ds(k * B_TILE, B_TILE)], start=False, stop=(k == NCH - 1))

        def load_x(it, slot):
            boff = it * B_TILE
            xt = xt_pool.tile([DA, B_TILE], MMDT, tag=f"xts{slot}", name="xt")
            nc.scalar.dma_start(xt[:], X[:, ds(boff, B_TILE)])
            return xt

        AB = agran * B_TILE  # ACT/DVE drain granularity (banks per op)
        NAG = NCH // agran

        def h_init(xt, slot):
            """h = silu(W_in^T x + b) for all 4 chunks: 4 matmuls into one
            wide PSUM tile, then sigmoid + (ps)*(s) mul per drain-granule
            so the drain pipelines behind the accumulation."""
            ps = bps_tile("ps_h")
            s = g_pool.tile([128, WB], MMDT, tag=f"o1s{slot}", name="s_h")
            h = h_pool.tile([128, WB], MMDT, tag=f"hs{slot}", name="h0")
            for m in range(NCH):
                mm(ps[:, ds(m * B_TILE, B_TILE)], win_sb[:, ts(m, 128)], xt[:],
                   start=True, stop=True)
            for a in range(NAG):
                sl = ds(a * AB, AB)
                nc.scalar.activation(s[:, sl], ps[:, sl], AF.Sigmoid)
                nc.vector.tensor_mul(h[:, sl], ps[:, sl], s[:, sl])
            return h

        def layer(i, xt, h, slot):
            gt = {}
            for w_sb, u_sb, fn, nm in gates:
                ps = bps_tile(f"ps_{nm}")
                gate_mms(ps, w_sb[i], u_sb[i], xt, h)
                t = g_pool.tile([128, WB], MMDT, tag=f"{nm}s{slot}", name="gate")
                for a in range(NAG):
                    sl = ds(a * AB, AB)
                    nc.scalar.activation(t[:, sl], ps[:, sl], fn)
                gt[nm] = t
                if nm == "o1":
                    # g = u * o1 in place over u's tile (u dead after)
                    ew_eng.tensor_mul(gt["u"][:], gt["u"][:], t[:])
            g = gt["u"]
            ps = bps_tile("ps_o2")
            for m in range(NCH):
                sub = ps[:, ds(m * B_TILE, B_TILE)]
                # bo2 rides a K=1 matmul against xt's ones row, so the o2
                # drain below is bias-free and granule-wide like the gates'.
                mm(sub, bo2_sb[i][0:1, ts(m, 128)], xt[0:1, :],
                   start=True, stop=False)
                for k in range(NCH):
                    mm(sub, wo2_sb[i][:, k * NN + m * 128: k * NN + (m + 1) * 128],
                       g[:, ds(k * B_TILE, B_TILE)], start=False, stop=(k == NCH - 1))
            # o2 = silu(ps) = ps * sigmoid(ps), per drain-granule; then
            # h' = f*h + o2 per granule too, so the next consumer of h'
            # (next layer's u-gate / the out-stage) starts on granule 0
            # instead of waiting for the whole-width update.
            s = g_pool.tile([128, WB], MMDT, tag=f"o1s{slot}", name="s_o2")
            fh = gt["f"]
            hn = h_pool.tile([128, WB], MMDT, tag=f"hs{slot}", name="hn")
            for a in range(NAG):
                sl = ds(a * AB, AB)
                nc.scalar.activation(s[:, sl], ps[:, sl], AF.Sigmoid)
                nc.vector.tensor_mul(s[:, sl], ps[:, sl], s[:, sl])
                # f*h in place over f's tile (f dead after this)
                nc.vector.tensor_mul(fh[:, sl], fh[:, sl], h[:, sl])
                nc.vector.tensor_add(hn[:, sl], fh[:, sl], s[:, sl])
            return hn

        def out_stage(it, h, slot):
            boff = it * B_TILE
            ps = bps_tile("ps_out")
            po = ps[0:1, ds(0, B_TILE)]
            for k in range(NCH):
                mm(po, wout_sb[:, k:k + 1], h[:, ds(k * B_TILE, B_TILE)],
                   start=(k == 0), stop=(k == NCH - 1))
            so = o_pool.tile([1, B_TILE], F32, tag=f"sos{slot}", name="so")
            nc.scalar.activation(so[:], po, AF.Sigmoid, bias=bout_sb[0:1, 0:1])
            nc.vector.scalar_tensor_tensor(
                so[:], po, bout_sb[0:1, 0:1], so[:], op0=ALU.add, op1=ALU.mult)
            nc.sync.dma_start(out[:, ds(boff, B_TILE)], so[:])

        def pipelined_pass():
            # nway tiles interleaved phase-by-phase so each tile's ACT/DVE
            # tail hides behind the other tiles' matmuls; across group
            # boundaries, the next group's X DMA is prefetched during layer 0
            # and its h_init matmuls are emitted between the out-stages
            # so PE never drains on the out/h-handoff.
            n_groups = n_tiles // nway
            xs = [load_x(s, s) for s in range(nway)]
            hs = [h_init(xs[s], s) for s in range(nway)]
            for g in range(n_groups):
                xn = None
                for i in range(L):
                    for s in range(nway):
                        hs[s] = layer(i, xs[s], hs[s], s)
                    if i == 0 and g + 1 < n_groups:
                        xn = [load_x((g + 1) * nway + s, s) for s in range(nway)]
                for s in range(nway):
                    out_stage(g * nway + s, hs[s], s)
                    if g + 1 < n_groups:
                        hs[s] = h_init(xn[s], s)
                if xn is not None:
                    xs = xn

        def body_pair(it_a, it_b):
            xa = load_x(it_a, 0)
            xb = load_x(it_b, 1)
            ha = h_init(xa, 0)
            hb = h_init(xb, 1)
            for i in range(L):
                ha = layer(i, xa, ha, 0)
                hb = layer(i, xb, hb, 1)
            out_stage(it_a, ha, 0)
            out_stage(it_b, hb, 1)

        for pi in range(n_passes):
            if loop_mode == "static":
                pipelined_pass()
            else:
                with tc.For_i(0, n_tiles // 2, 1, hint_engines=(mybir.EngineType.PE,),
                              staggered_reset=staggered) as it:
                    body_pair(it * 2, it * 2 + 1)

    nc.compile()
    return nc


_NC_CACHE = {}


def _get_nc(n_passes=1, mm_dtype="bf16", **kw):
    key = (n_passes, mm_dtype, tuple(sorted(kw.items())))
    if key not in _NC_CACHE:
        _NC_CACHE[key] = build_kernel(n_passes, mm_dtype, **kw)
    return _NC_CACHE[key]


def make_in_maps(inputs: dict):
    """Shard X (host-transposed, ones-augmented, bf16, per core); fold each
    input-side bias into its weight as an extra row; weights bf16,
    replicated by reference (no per-core host copies)."""
    bf = mybir.dt.np(BF16)

    def f32(k):
        return np.ascontiguousarray(np.asarray(inputs[k], dtype=np.float32))

    def aug(wk, bk, axis):
        # bias row FIRST (matmul rhs base partition must be 0 for the
        # ones-row K=1 bias matmuls, so X's ones row is row 0 too)
        return np.concatenate([f32(bk), f32(wk)], axis=axis).astype(bf)

    shared = {
        "W_in": aug("W_in", "b_in", 0),
        "Wf": aug("Wf", "bf", 1),
        "Wu": aug("Wu", "bu", 1),
        "Wo1": aug("Wo1", "bo1", 1),
        "Uf": f32("Uf").astype(bf),
        "Uu": f32("Uu").astype(bf),
        "Uo1": f32("Uo1").astype(bf),
        "Wo2": f32("Wo2").astype(bf),
        "bo2": f32("bo2").astype(bf),
        "W_out": f32("W_out").astype(bf),
        "b_out": f32("b_out"),
    }
    X = np.asarray(inputs["X"])
    ones = np.ones((1, B_CORE), np.float32)
    return [
        {"X": np.concatenate(
            [ones, np.asarray(X[c * B_CORE:(c + 1) * B_CORE], np.float32).T],
            axis=0).astype(bf), **shared}
        for c in range(N_CORES)
    ]


_HELD = {}


def _make_held(nc):
    """Persistent jitted shard_map callable over a prebuilt Bass module
    (mirrors bass2jax.run_bass_via_pjrt, but reusable so the NEFF stays
    loaded on the devices across calls)."""
    import jax
    from jax.sharding import Mesh, NamedSharding, PartitionSpec
    from jax.experimental.shard_map import shard_map
    from concourse.bass2jax import (
        _bass_exec_p, install_neuronx_cc_hook, partition_id_tensor)

    install_neuronx_cc_hook()
    partition_name = nc.partition_id_tensor.name if nc.partition_id_tensor else None
    in_names, out_names, out_avals, zero_outs = [], [], [], []
    for alloc in nc.m.functions[0].allocations:
        if not isinstance(alloc, mybir.MemoryLocationSet):
            continue
        name = alloc.memorylocations[0].name
        if alloc.kind == "ExternalInput":
            if name != partition_name:
                in_names.append(name)
        elif alloc.kind == "ExternalOutput":
            shape = tuple(alloc.tensor_shape)
            dtype = mybir.dt.np(alloc.dtype)
            out_names.append(name)
            out_avals.append(jax.core.ShapedArray(shape, dtype))
            zero_outs.append(np.zeros(shape, dtype))
    n_params = len(in_names)
    all_in = list(in_names) + list(out_names)
    if partition_name is not None:
        all_in.append(partition_name)

    def _body(*args):
        operands = list(args)
        if partition_name is not None:
            operands.append(partition_id_tensor())
        return tuple(_bass_exec_p.bind(
            *operands, out_avals=tuple(out_avals), in_names=tuple(all_in),
            out_names=tuple(out_names), lowering_input_output_aliases=(),
            sim_require_finite=True, sim_require_nnan=True, nc=nc))

    devices = jax.devices()[:N_CORES]
    mesh = Mesh(np.asarray(devices), ("core",))
    nio = n_params + len(out_names)
    fn = jax.jit(shard_map(_body, mesh=mesh,
                           in_specs=(PartitionSpec("core"),) * nio,
                           out_specs=(PartitionSpec("core"),) * len(out_names),
                           check_rep=False), keep_unused=True)
    sh = NamedSharding(mesh, PartitionSpec("core"))
    return fn, in_names, out_names, zero_outs, sh


_DEV_CACHE = {}


def _dev_put_cached(key, arr, sh):
    """device_put with a byte-compare cache: re-uploading 50+ MB of
    replicated weights dominates a warm call, while comparing host bytes
    costs ~ms. Falls through to a fresh upload whenever content changes."""
    import jax
    ent = _DEV_CACHE.get(key)
    if (ent is not None and ent[0].shape == arr.shape
            and ent[0].dtype == arr.dtype and np.array_equal(ent[0], arr)):
        return ent[1]
    dev = jax.device_put(arr, sh)
    _DEV_CACHE[key] = (arr, dev)
    return dev


def _run_held(nc, in_maps):
    import jax
    key = id(nc)
    if key not in _HELD:
        _HELD[key] = _make_held(nc)
    fn, in_names, out_names, zero_outs, sh = _HELD[key]
    dev_in = [
        _dev_put_cached(
            (key, nm),
            np.concatenate([in_maps[c][nm] for c in range(N_CORES)], axis=0), sh)
        for nm in in_names]
    dev_in += [
        _dev_put_cached(
            (key, f"_z{j}"),
            np.zeros((N_CORES * z.shape[0], *z.shape[1:]), z.dtype), sh)
        for j, z in enumerate(zero_outs)]
    outs = jax.device_get(fn(*dev_in))
    return [{nm: np.asarray(outs[j]).reshape(N_CORES, *z.shape)[c]
             for j, (nm, z) in enumerate(zip(out_names, zero_outs))}
            for c in range(N_CORES)]


def run(inputs: dict, n_passes: int = 1, mm_dtype: str = "bf16", **kw):
    """Shard, run on 8 cores, gather. Returns (full_output, results_list)."""
    nc = _get_nc(n_passes, mm_dtype, **kw)
    in_maps = make_in_maps(inputs)
    try:
        results = _run_held(nc, in_maps)
    except Exception:
        results = run_bass_kernel_spmd(
            nc, in_maps, core_ids=list(range(N_CORES))).results
    full = np.concatenate(
        [results[c]["out"].reshape(B_CORE, 1) for c in range(N_CORES)], axis=0)
    return full, results


def kernel(**inputs) -> np.ndarray:
    full, _ = run(inputs)
    return full
